# revision 13
# baseline (speedup 1.0000x reference)
"""2-layer dense GCN on 8 Trainium2 NeuronCores.

Reference computation (all fp32):
    H0 = relu((A_norm @ X) @ W0)
    H1 = relu((A_norm @ H0) @ W1)
A_norm: [16384, 16384], X: [16384, 128], W0/W1: [128, 128].

Sharding: 1D row partition of A_norm (2048 rows/core). Each core holds
A[rows_c].T (host-transposed so the node-contraction dim lands on SBUF
partitions), computes its row block of each layer, and the hidden state
is exchanged between layers with chunked on-device AllGathers.

Device layout is transpose-free:
  - aggregate:  psum[d, i] += X_tile[j, d].T @ A_T_tile[j, i]
                (lhsT = stationary node-major X/H tile, rhs = A^T slice)
  - linear:     psum[i, e]  = M^T_tile[d, i].T @ W[d, e]   (node-major out)
  - relu fused into the PSUM->SBUF eviction on the scalar engine.

The aggregation runs CHUNK-MAJOR (one 512-wide output chunk at a time,
full contraction each): chunk k's hidden tiles finish at ~(k+1)/4 of the
layer, so AllGather k overlaps the remaining chunks' compute — only the
last AllGather is exposed at the layer boundary. The stationary H layout
in SBUF ([128, 512] pieces) is exactly what the chunked AllGathers
produce, so no transposes are needed anywhere.

PRECISION modes:
  - "fp32":   exact fp32 matmuls (4 cyc/row on the PE).
  - "split3": A and X/H split into bf16 hi+lo; aggregate computed as
              Ah@Xh + Al@Xh + Ah@Xl (3 bf16 passes, ~2.5e-6 rel err —
              fp32-class).
  - "bf16":   plain bf16 aggregate (1 cyc/row, half the DMA bytes,
              ~1.1e-3 rel err).
"""

import sys
from contextlib import ExitStack

if "/opt/trn_rl_repo" not in sys.path:
    sys.path.insert(0, "/opt/trn_rl_repo")

import numpy as np

N_NODES = 16384
D = 128
NCORES = 8
ROWS = N_NODES // NCORES  # 2048

PRECISION = "fp32"  # "fp32" | "split3" | "bf16"


def _geom(n_nodes=N_NODES, ncores=NCORES, precision=PRECISION):
    esz = 4 if precision == "fp32" else 2
    nsplit = 2 if precision == "split3" else 1  # hi/lo operand copies
    rows = n_nodes // ncores
    jt = n_nodes // 128          # total j-tiles (contraction tiles)
    jt_per_rank = jt // ncores   # j-tiles covered by one rank's nodes
    ic = min(512, rows)          # i-chunk width (one PSUM bank, fp32 out)
    nch = rows // ic             # i-chunks per core
    # j-tiles per A DMA: target ~4 MiB per transfer (2 MiB per split
    # stream for split3 so the hi+lo pools fit in SBUF)
    target = (2 if nsplit == 2 else 4) * 1024 * 1024
    jg = max(1, target // (128 * ic * esz))
    jg = min(jg, jt)
    while jt % jg:
        jg -= 1
    return dict(
        esz=esz, nsplit=nsplit, rows=rows, jt=jt, jt_per_rank=jt_per_rank,
        ic=ic, nch=nch, jg=jg, ndma_pc=jt // jg,
        a_bufs=4 if nsplit == 1 else 3,
    )


def build_gcn(n_nodes=N_NODES, d=D, ncores=NCORES, precision=PRECISION):
    """Build the SPMD Bass program (one program, runs on all cores)."""
    import concourse.bass as bass  # noqa: F401
    import concourse.tile as tile
    from concourse import bacc, mybir

    F32 = mybir.dt.float32
    BF16 = mybir.dt.bfloat16
    agg_dt = F32 if precision == "fp32" else BF16

    g_ = _geom(n_nodes, ncores, precision)
    nsplit, rows, jt = g_["nsplit"], g_["rows"], g_["jt"]
    jt_per_rank, ic, nch = g_["jt_per_rank"], g_["ic"], g_["nch"]
    jg, ndma_pc, a_bufs = g_["jg"], g_["ndma_pc"], g_["a_bufs"]
    lt = ic // 128               # linear i-tiles (and h tiles) per chunk
    jt_pp = jt_per_rank // nch   # j-tiles per stationary piece

    nc = bacc.Bacc("TRN2", target_bir_lowering=False, num_devices=ncores)

    # A^T shards, host pre-tiled chunk-major: DMA group (c, g) is the
    # contiguous block a_in[(c*ndma_pc+g)*128 : +128, :], covering
    # j-tiles [g*jg, (g+1)*jg) x output columns [c*ic, (c+1)*ic)
    a_in = [
        nc.dram_tensor(
            f"a{s}", [nch * ndma_pc * 128, jg * ic], agg_dt, kind="ExternalInput"
        )
        for s in range(nsplit)
    ]
    # x_t: X pre-tiled on host into the AllGather layout:
    # x_t[r*128 + p, tl*128 + dd] = X[(r*jt_per_rank + tl)*128 + p, dd]
    x_in = [
        nc.dram_tensor(f"x{s}", [ncores * 128, rows], agg_dt, kind="ExternalInput")
        for s in range(nsplit)
    ]
    w0 = nc.dram_tensor("w0", [d, d], F32, kind="ExternalInput")
    w1 = nc.dram_tensor("w1", [d, d], F32, kind="ExternalInput")
    h_out = nc.dram_tensor("h_out", [rows, d], F32, kind="ExternalOutput")

    relu = mybir.ActivationFunctionType.Relu

    with tile.TileContext(nc) as tc, ExitStack() as ctx:
        sb1 = ctx.enter_context(tc.tile_pool(name="sb1", bufs=1))
        stat_pool = ctx.enter_context(
            tc.tile_pool(
                name="stat",
                bufs=ncores * nch * nsplit + (16 if agg_dt != F32 else 0),
            )
        )
        a_pool = ctx.enter_context(tc.tile_pool(name="a", bufs=a_bufs))
        m_pool = ctx.enter_context(tc.tile_pool(name="m", bufs=2))
        h_pool = ctx.enter_context(tc.tile_pool(name="h", bufs=4))
        split_pool = ctx.enter_context(tc.tile_pool(name="spl", bufs=4))
        agg_pool = ctx.enter_context(tc.tile_pool(name="agg", bufs=2, space="PSUM"))
        lin_pool = ctx.enter_context(tc.tile_pool(name="lin", bufs=2, space="PSUM"))
        dram = ctx.enter_context(tc.tile_pool(name="dram", bufs=1, space="DRAM"))

        w0_sb = sb1.tile([d, d], F32)
        nc.scalar.dma_start(out=w0_sb[:], in_=w0[:])
        w1_sb = sb1.tile([d, d], F32)
        nc.scalar.dma_start(out=w1_sb[:], in_=w1[:])

        def load_stat_pieces(srcs, lname):
            """srcs: per split s, per piece k: [ncores*128, ic] DRAM views.
            Returns stat[s][r][k] = [128, ic] SBUF tile."""
            out = []
            for s in range(nsplit):
                per_rank = [[None] * nch for _ in range(ncores)]
                for k in range(nch):
                    src = srcs[s][k]
                    for r in range(ncores):
                        sc = stat_pool.tile(
                            [128, ic], agg_dt, name=f"{lname}{s}_{r}_{k}", tag="sc"
                        )
                        nc.scalar.dma_start(
                            out=sc[:], in_=src[r * 128 : (r + 1) * 128, :]
                        )
                        per_rank[r][k] = sc
                out.append(per_rank)
            return out

        def layer(stat, w_sb, write_out, chunk_done):
            # stat[s][r][k]: stationary pieces; j-tile j lives in piece
            # (r=j//jt_per_rank, k=(j%jt_per_rank)//jt_pp) col (j%jt_pp)
            passes = [(0, 0)] if nsplit == 1 else [(0, 0), (1, 0), (0, 1)]
            for c in range(nch):
                ps = agg_pool.tile([128, ic], F32, name=f"ps{c}", tag="ps")
                for g in range(ndma_pc):
                    ats = []
                    for s in range(nsplit):
                        at = a_pool.tile(
                            [128, jg * ic], agg_dt, name=f"at{s}", tag=f"at{s}"
                        )
                        nc.sync.dma_start(
                            out=at[:],
                            in_=a_in[s][
                                (c * ndma_pc + g) * 128 : (c * ndma_pc + g + 1) * 128,
                                :,
                            ],
                        )
                        ats.append(at)
                    for t in range(jg):
                        j = g * jg + t
                        r, jr = j // jt_per_rank, j % jt_per_rank
                        piece = jr // jt_pp
                        tl2 = jr % jt_pp
                        for pi, (ls, rs) in enumerate(passes):
                            lhs = stat[ls][r][piece][
                                :, tl2 * 128 : (tl2 + 1) * 128
                            ]
                            nc.tensor.matmul(
                                ps[:],
                                lhsT=lhs,
                                rhs=ats[rs][:, t * ic : (t + 1) * ic],
                                start=(j == 0 and pi == 0),
                                stop=(j == jt - 1 and pi == len(passes) - 1),
                            )
                # linear + relu, node-major output tiles
                mt = m_pool.tile([128, ic], F32, name="mt", tag="mt")
                nc.vector.tensor_copy(out=mt[:], in_=ps[:])
                for it in range(lt):
                    lp = lin_pool.tile([128, d], F32, name="lp", tag="lp")
                    nc.tensor.matmul(
                        lp[:],
                        lhsT=mt[:, it * 128 : (it + 1) * 128],
                        rhs=w_sb[:],
                        start=True,
                        stop=True,
                    )
                    ht = h_pool.tile([128, d], F32, name="ht", tag="ht")
                    nc.scalar.activation(ht[:], lp[:], relu)
                    write_out(c, it, ht)
                chunk_done(c)

        # ---- layer 0 ----
        stat0 = load_stat_pieces(
            [
                [
                    x_in[s][:, k * ic : (k + 1) * ic]
                    for k in range(nch)
                ]
                for s in range(nsplit)
            ],
            "sx",
        )
        # per-chunk hidden-state bounce + chunked AllGathers ([hi | lo])
        h_tb = [
            dram.tile([128, nsplit * ic], agg_dt, name=f"h_tb{k}") for k in range(nch)
        ]
        h_ag = [
            dram.tile(
                [ncores * 128, nsplit * ic], agg_dt, addr_space="Shared",
                name=f"h_ag{k}",
            )
            for k in range(nch)
        ]

        def write_l0(c, it, ht):
            if precision == "fp32":
                nc.scalar.dma_start(
                    out=h_tb[c][:, it * 128 : (it + 1) * 128], in_=ht[:]
                )
                return
            hh = split_pool.tile([128, d], BF16, name="hh", tag="hh")
            nc.vector.tensor_copy(out=hh[:], in_=ht[:])
            nc.scalar.dma_start(out=h_tb[c][:, it * 128 : (it + 1) * 128], in_=hh[:])
            if nsplit == 2:
                hh32 = split_pool.tile([128, d], F32, name="hh32", tag="hh32")
                nc.vector.tensor_copy(out=hh32[:], in_=hh[:])
                hl = split_pool.tile([128, d], BF16, name="hl", tag="hl")
                nc.vector.tensor_sub(out=hl[:], in0=ht[:], in1=hh32[:])
                nc.scalar.dma_start(
                    out=h_tb[c][:, ic + it * 128 : ic + (it + 1) * 128], in_=hl[:]
                )

        def ag_l0(c):
            import concourse.mybir as _mb

            nc.gpsimd.collective_compute(
                "AllGather",
                _mb.AluOpType.bypass,
                replica_groups=[list(range(ncores))],
                ins=[h_tb[c][:]],
                outs=[h_ag[c][:]],
            )

        layer(stat0, w0_sb, write_l0, ag_l0)

        # ---- layer 1 ----
        stat1 = load_stat_pieces(
            [
                [h_ag[k][:, s * ic : (s + 1) * ic] for k in range(nch)]
                for s in range(nsplit)
            ],
            "sh",
        )

        def write_l1(c, it, ht):
            nc.scalar.dma_start(
                out=h_out[c * ic + it * 128 : c * ic + (it + 1) * 128, :], in_=ht[:]
            )

        layer(stat1, w1_sb, write_l1, lambda c: None)

    nc.finalize()
    return nc


def _tile_stat(X, ncores, jt_per_rank):
    rows = jt_per_rank * 128
    return np.ascontiguousarray(
        X.reshape(ncores, jt_per_rank, 128, D).transpose(0, 2, 1, 3)
        .reshape(ncores * 128, rows)
    )


def shard_inputs(A_norm, X, n_nodes=N_NODES, ncores=NCORES, precision=PRECISION):
    """Host-side shard prep. Returns per-core input maps."""
    import ml_dtypes

    bf16 = ml_dtypes.bfloat16
    g_ = _geom(n_nodes, ncores, precision)
    rows, jt_per_rank = g_["rows"], g_["jt_per_rank"]
    ic, nch, jg, ndma_pc = g_["ic"], g_["nch"], g_["jg"], g_["ndma_pc"]

    def tile_a(a_tc):
        # [n_nodes, rows] -> chunk-major [nch*ndma_pc*128, jg*ic] with
        # a_pre[(c*ndma_pc+g)*128+p, t*ic+i] = a_tc[(g*jg+t)*128+p, c*ic+i]
        return np.ascontiguousarray(
            a_tc.reshape(ndma_pc, jg, 128, nch, ic)
            .transpose(3, 0, 2, 1, 4)
            .reshape(nch * ndma_pc * 128, jg * ic)
        )

    x_t = _tile_stat(X, ncores, jt_per_rank)
    if precision == "fp32":
        xs = [x_t]
    else:
        x_hi = x_t.astype(bf16)
        xs = [x_hi]
        if precision == "split3":
            xs.append((x_t - x_hi.astype(np.float32)).astype(bf16))

    in_maps = []
    for c in range(ncores):
        a_tc = np.ascontiguousarray(A_norm[c * rows : (c + 1) * rows, :].T)
        m = {}
        if precision == "fp32":
            m["a0"] = tile_a(a_tc)
        else:
            a_hi = a_tc.astype(bf16)
            m["a0"] = tile_a(a_hi)
            if precision == "split3":
                m["a1"] = tile_a((a_tc - a_hi.astype(np.float32)).astype(bf16))
        for s, x in enumerate(xs):
            m[f"x{s}"] = x
        in_maps.append(m)
    return in_maps


_CACHED = {}


def kernel(A_norm, X, W0, W1):
    A_norm = np.ascontiguousarray(A_norm, dtype=np.float32)
    X = np.ascontiguousarray(X, dtype=np.float32)
    W0 = np.ascontiguousarray(W0, dtype=np.float32)
    W1 = np.ascontiguousarray(W1, dtype=np.float32)

    from concourse.bass_utils import run_bass_kernel_spmd

    if PRECISION not in _CACHED:
        _CACHED[PRECISION] = build_gcn(precision=PRECISION)
    nc = _CACHED[PRECISION]

    in_maps = shard_inputs(A_norm, X, precision=PRECISION)
    for m in in_maps:
        m["w0"] = W0
        m["w1"] = W1

    res = run_bass_kernel_spmd(nc, in_maps, core_ids=list(range(NCORES)))
    return np.concatenate([res.results[c]["h_out"] for c in range(NCORES)], axis=0)
